# revision 58
# baseline (speedup 1.0000x reference)
"""Trainium2 Bass kernel for nn_CustomLlamaModel (2-layer MQA llama, B=1 S=2048
H=1024 HQ=16 HKV=1 FF=4096), sequence-parallel over 8 NeuronCores.

Each core owns 256 tokens end-to-end; all weights are streamed to every core
from its own HBM copy (host-packed into SBUF-image layouts so every weight
tile is a single 2D DMA).  With no causal mask and one shared KV head the only
inter-core traffic is a 64KB AllGather per layer (roped K head-dim-major + V
token-major).  Key scheduling ideas:
  - attention inner loop is software-pipelined: scores(k+1) is emitted before
    attn@v(k), so the PE never sits on the exp(k) semaphore and back-to-back
    matmul overlap (173ns drain) is preserved; attn@v alternates between two
    PSUM accumulators.
  - MLP is two-phase: all gate/up chunks produce a full fp act buffer, then
    down-proj accumulates 32 f-tiles per output tile fully in PSUM (8 banks),
    one residual add per tile instead of per-chunk adds.
  - rmsnorm rsqrt = exp(-0.5*ln(ms+eps)) on ACT (ln/exp share a table with
    attention's exp -> no table reloads mid-layer); softmax denominators use
    the fast custom-DVE reciprocal.
  - sum-of-squares for the next rmsnorm accumulates on the PE inline with the
    residual adds.
"""
import sys

sys.path.insert(0, "/opt/trn_rl_repo")

import ml_dtypes
import numpy as np
import orjson

import concourse.bass as bass
import concourse.mybir as mybir
import concourse.tile as tile
from concourse import bass_utils
from concourse.masks import make_identity

# ---------------------------------------------------------------------------
# Walrus in this container supports only ONE sync-wait per instruction, but
# Tile's scheduler emits multi-wait instructions.  Post-process the BIR JSON:
# split each multi-wait instruction into single-wait NoOps (same engine,
# program-order before the original).
# ---------------------------------------------------------------------------
_orig_to_json_bytes = bass.Bass.to_json_bytes
_MW = [0]


def _split_multiwait(d):
    changed = False

    def fix_block(bb):
        nonlocal changed
        insts = bb.get("instructions")
        if not insts:
            return
        out = []
        for ins in insts:
            si = ins.get("sync_info")
            if si:
                ow = si.get("on_wait") or []
                if len(ow) > 1:
                    changed = True
                    for w in ow[:-1]:
                        _MW[0] += 1
                        out.append({
                            "debug": ins.get("debug", 0),
                            "engine": ins["engine"],
                            "ins": [],
                            "outs": [],
                            "name": f"{ins['name']}-mw{_MW[0]}",
                            "opcode": "NoOp",
                            "sync_info": {"on_update": [], "on_wait": [w]},
                        })
                    si["on_wait"] = [ow[-1]]
            out.append(ins)
        bb["instructions"] = out

    def rec(o):
        if isinstance(o, dict):
            if isinstance(o.get("instructions"), list):
                fix_block(o)
            for v in o.values():
                rec(v)
        elif isinstance(o, list):
            for v in o:
                rec(v)

    for fn in d.get("functions", []):
        rec(fn)
    return changed


def _patched_to_json_bytes(self):
    raw = _orig_to_json_bytes(self)
    d = orjson.loads(raw)
    if _split_multiwait(d):
        return orjson.dumps(d)
    return raw


bass.Bass.to_json_bytes = _patched_to_json_bytes



# ---------------------------------------------------------------------------
# Model / sharding constants
# ---------------------------------------------------------------------------
S, H, D, HQ, FF, L, V = 2048, 1024, 64, 16, 4096, 2, 32000
EPS = 1e-6
NCORES = 8
TOK = S // NCORES       # tokens per core (256)
HT = H // 128           # 8 hidden feature tiles
KT = S // 128           # 16 key-token tiles
NGRP = 4                # attention head groups
HPG = HQ // NGRP        # heads per group (4)
MCH = 8                 # mlp ff chunks (gate/up streaming granularity)
CHF = FF // MCH         # 512 ff per chunk
FPC = CHF // 128        # 4 ff tiles per chunk
FT = FF // 128          # 32 ff tiles total
F32 = mybir.dt.float32
F32R = mybir.dt.float32r
BF16 = mybir.dt.bfloat16
MUL = mybir.AluOpType.mult
ADD = mybir.AluOpType.add
RG = [list(range(NCORES))]
BF = ml_dtypes.bfloat16
AF = mybir.ActivationFunctionType

_CACHED_NC = None


def _build_nc():
    nc = bass.Bass()
    # host-packed SBUF-image layouts: one contiguous 2D DMA per tile
    x0 = nc.dram_tensor("x0", [128, HT * TOK], F32, kind="ExternalInput")
    cosq = nc.dram_tensor("cosq", [64, HPG * TOK], BF16, kind="ExternalInput")
    sinq = nc.dram_tensor("sinq", [64, HPG * TOK], BF16, kind="ExternalInput")
    W = []
    for l in range(L):
        W.append({
            "wq": nc.dram_tensor(f"wq{l}", [128, HT * H], BF16,
                                 kind="ExternalInput"),
            "wkv": nc.dram_tensor(f"wkv{l}", [128, HT * 128], F32,
                                  kind="ExternalInput"),
            "wo": nc.dram_tensor(f"wo{l}", [128, HT * H], BF16,
                                 kind="ExternalInput"),
            "wg": nc.dram_tensor(f"wg{l}", [128, MCH * HT * CHF], BF16,
                                 kind="ExternalInput"),
            "wu": nc.dram_tensor(f"wu{l}", [128, MCH * HT * CHF], BF16,
                                 kind="ExternalInput"),
            "wd": nc.dram_tensor(f"wd{l}", [128, FT * H], BF16,
                                 kind="ExternalInput"),
        })
    xout = nc.dram_tensor("xout", [128, HT * TOK], F32, kind="ExternalOutput")
    import os
    DBG = bool(os.environ.get("KDBG"))
    if DBG:
        dbg = {
            "d_h1": nc.dram_tensor("d_h1", [128, HT * TOK], BF16,
                                   kind="ExternalOutput"),
            "d_kvp": nc.dram_tensor("d_kvp", [128, 256], BF16,
                                    kind="ExternalOutput"),
            "d_q2": nc.dram_tensor("d_q2", [128, HQ * TOK], BF16,
                                   kind="ExternalOutput"),
            "d_oT": nc.dram_tensor("d_oT", [128, HT * TOK], BF16,
                                   kind="ExternalOutput"),
            "d_x1": nc.dram_tensor("d_x1", [128, HT * TOK], F32,
                                   kind="ExternalOutput"),
            "d_act": nc.dram_tensor("d_act", [128, FT * TOK], BF16,
                                    kind="ExternalOutput"),
            "d_vtok": nc.dram_tensor("d_vtok", [128, KT * 65], BF16,
                                     kind="ExternalOutput"),
            "d_kT": nc.dram_tensor("d_kT", [128, (KT // 2) * 128], BF16,
                                   kind="ExternalOutput"),
        }

    with tile.TileContext(nc) as tc:
        with (
            tc.tile_pool(name="const", bufs=1) as pconst,
            tc.tile_pool(name="resid", bufs=1) as presid,
            tc.tile_pool(name="attw", bufs=2) as pattw,
            tc.tile_pool(name="mlpw", bufs=2) as pmlpw,
            tc.tile_pool(name="dw", bufs=2) as pdw,
            tc.tile_pool(name="acts", bufs=1) as pacts,
            tc.tile_pool(name="big", bufs=1) as pbig,
            tc.tile_pool(name="small", bufs=2) as psmall,
            tc.tile_pool(name="exp", bufs=3) as pexp,
            tc.tile_pool(name="dram", bufs=2, space="DRAM") as pdram,
        ):
            # constants
            identf = pconst.tile([128, 128], F32, tag="identf")
            make_identity(nc, identf[:])
            ident = pconst.tile([128, 128], BF16, tag="ident")
            nc.vector.tensor_copy(ident[:], identf[:])
            onesf = pconst.tile([128, 128], F32, tag="onesf")
            nc.vector.memset(onesf[:], 1.0)
            ones = pconst.tile([128, 128], BF16, tag="ones")
            nc.vector.tensor_copy(ones[:], onesf[:])
            onesr = pconst.tile([128, 128], F32R, tag="onesr")
            nc.vector.tensor_copy(onesr[:], onesf[:])
            epst = pconst.tile([128, 1], F32, tag="eps")
            nc.gpsimd.memset(epst[:], EPS)
            # residual x, feature-major, one tile per 128-feature block so
            # consumers wait only on the block they read (dep tracking is
            # tile-granular)
            xs = []
            for j in range(HT):
                xt = presid.tile([128, TOK], F32, tag=f"x{j}",
                                 name=f"x_{j}")
                nc.sync.dma_start(xt[:], x0[:, j * TOK:(j + 1) * TOK])
                xs.append(xt)
            cos_sb = pconst.tile([64, HPG * TOK], BF16, tag="cos")
            sin_sb = pconst.tile([64, HPG * TOK], BF16, tag="sin")
            nc.sync.dma_start(cos_sb[:], cosq[:])
            nc.sync.dma_start(sin_sb[:], sinq[:])

            def load_attw(l):
                """One DMA per attention weight tensor (ring of 2)."""
                wq_sb = pattw.tile([128, HT * H], BF16, tag="wq")
                wkv_sb = pattw.tile([128, HT * 128], F32, tag="wkv",
                                    bufs=1)
                wo_sb = pattw.tile([128, HT * H], BF16, tag="wo")
                nc.sync.dma_start(wq_sb[:], W[l]["wq"][:])
                nc.sync.dma_start(wkv_sb[:], W[l]["wkv"][:])
                nc.sync.dma_start(wo_sb[:], W[l]["wo"][:])
                return wq_sb, wkv_sb, wo_sb

            def rope(src, ncols, c0):
                """In-place RoPE on src[:, c0:c0+ncols] ([64, ncols] head-dim
                major; cos/sin tables carry the sign pattern and repeat every
                TOK cols, table width HPG*TOK)."""
                for w0 in range(0, ncols, HPG * TOK):
                    wn = min(HPG * TOK, ncols - w0)
                    cs = slice(c0 + w0, c0 + w0 + wn)
                    rot = psmall.tile([64, HPG * TOK], BF16,
                                      tag="ropescratch", bufs=2,
                                      name=f"rot{c0}_{w0}")
                    nc.vector.tensor_copy(rot[0:32, 0:wn], src[32:64, cs])
                    nc.vector.tensor_copy(rot[32:64, 0:wn], src[0:32, cs])
                    nc.vector.tensor_tensor(
                        rot[:, 0:wn], rot[:, 0:wn], sin_sb[:, 0:wn], op=MUL)
                    nc.vector.tensor_tensor(
                        src[:, cs], src[:, cs], cos_sb[:, 0:wn], op=MUL)
                    nc.vector.tensor_tensor(
                        src[:, cs], src[:, cs], rot[:, 0:wn], op=ADD)

            # --- rmsnorm split into: ssq accumulation (emitted inline with
            # residual adds by callers) and the tail (rsqrt + broadcast + h) ---
            def ssq_square(ppool, ssq, kf, tag):
                """xs[kf] -> square (DVE) -> PE ones-reduce into ssq."""
                x2 = psmall.tile([128, TOK], F32R, tag="xsq", bufs=3,
                                 name=f"x2_{tag}_{kf}")
                nc.vector.tensor_tensor(x2[:], xs[kf][:], xs[kf][:],
                                        op=MUL)
                nc.tensor.matmul(
                    ssq[:, 0:TOK], onesr[:, 0:1], x2[:],
                    start=(kf == 0), stop=(kf == HT - 1),
                )

            def rms_tail(ppool, ssq, tag):
                """ssq -> rinv = exp(-0.5*ln(ms+eps)) -> rb broadcast -> h
                (one tile per feature block)."""
                lnt = psmall.tile([1, TOK], F32, tag="lnt",
                                  name=f"ln_{tag}")
                nc.scalar.activation(lnt[:], ssq[:, 0:TOK], AF.Ln,
                                     bias=epst[0:1, :], scale=1.0 / H)
                rinv = psmall.tile([1, TOK], F32, tag="rinv",
                                   name=f"rinv_{tag}")
                nc.scalar.activation(rinv[:], lnt[:], AF.Exp, scale=-0.5)
                rinvr = psmall.tile([1, TOK], F32R, tag="rinvr",
                                    name=f"rinvr_{tag}")
                with nc.allow_low_precision(reason="f32r is fp32 bits"):
                    nc.vector.tensor_copy(rinvr[:], rinv[:])
                rb = ppool.tile([128, 512], F32, tag="rb", bufs=1)
                nc.tensor.matmul(rb[:, 0:TOK], onesr[0:1, :],
                                 rinvr[:], start=True, stop=True)
                # move rb off PSUM immediately: its bank gets recycled by the
                # next phase's pool, and keeping readers on it would give the
                # next phase's first matmul a WAR wait on the last h-mult
                rbsb = psmall.tile([128, TOK], F32, tag="rbsb", bufs=2,
                                   name=f"rbsb_{tag}")
                nc.scalar.copy(rbsb[:], rb[:, 0:TOK])
                hs = []
                for kf in range(HT):
                    ht_ = pbig.tile([128, TOK], BF16, tag=f"h{kf}",
                                    bufs=2, name=f"h_{tag}_{kf}")
                    nc.vector.tensor_tensor(ht_[:], xs[kf][:],
                                            rbsb[:], op=MUL)
                    hs.append(ht_)
                return hs, rbsb

            # layer-0 attention weights + first ssq
            attw_cur = load_attw(0)
            with tc.tile_pool(name="ps_n0", bufs=1, space="PSUM") as pps0:
                ssq0 = pps0.tile([1, 512], F32, tag="ssq0")
                for kf in range(HT):
                    ssq_square(pps0, ssq0, kf, "a0")
                h1, rb1 = rms_tail(pps0, ssq0, "a0")
            if DBG:
                for kf in range(HT):
                    nc.sync.dma_start(
                        dbg["d_h1"][:, kf * TOK:(kf + 1) * TOK], h1[kf][:])

            for l in range(L):
                w = W[l]
                wq_sb, wkv_sb, wo_sb = attw_cur

                with tc.tile_pool(name=f"ps_qkv{l}", bufs=2,
                                  space="PSUM") as pps:
                    # kv projection directly from x in fp32 (runs on the PE
                    # during the rmsnorm tail; the per-token rinv scale
                    # commutes with the feature contraction and is folded in
                    # at the copy below) -> [k(64)|v(64), TOK]
                    pkv = pps.tile([128, 512], F32, tag="pkv")
                    for kf in range(HT):
                        nc.tensor.matmul(
                            pkv[:, 0:TOK], wkv_sb[:, kf * 128:(kf + 1) * 128],
                            xs[kf][:],
                            start=(kf == 0), stop=(kf == HT - 1),
                        )
                    # kv_sb [64, 2*TOK]: cols 0:TOK = K, TOK:2*TOK = V
                    kv_sb = pbig.tile([64, 2 * TOK], BF16, tag="kv")
                    nc.vector.tensor_tensor(kv_sb[:, 0:TOK],
                                            pkv[0:64, 0:TOK],
                                            rb1[0:64, :], op=MUL)
                    nc.vector.tensor_copy(kv_sb[:, TOK:2 * TOK],
                                          pkv[64:128, 0:TOK])
                    nc.vector.tensor_tensor(kv_sb[:, TOK:2 * TOK],
                                            kv_sb[:, TOK:2 * TOK],
                                            rb1[0:64, :], op=MUL)
                    # rope K (local token positions = first TOK cols of tables)
                    rope(kv_sb, TOK, 0)
                    # pack: cols 0:128 = K hd-major (two 128-token halves
                    # stacked); cols 128:256 = V token-major (two transposes)
                    kvpack = pbig.tile([128, 2 * 128], BF16, tag="kvpack")
                    nc.vector.tensor_copy(kvpack[0:64, 0:128],
                                          kv_sb[0:64, 0:128])
                    nc.vector.tensor_copy(kvpack[64:128, 0:128],
                                          kv_sb[0:64, 128:256])
                    for t in range(2):
                        pvt = pps.tile([128, 1024], BF16, tag="pvt")
                        nc.tensor.transpose(
                            pvt[:, 0:64],
                            kv_sb[:, TOK + t * 128:TOK + (t + 1) * 128],
                            ident[0:64, 0:64],
                        )
                        nc.vector.tensor_copy(
                            kvpack[:, 128 + t * 64:128 + (t + 1) * 64],
                            pvt[:, 0:64])
                    if DBG and l == 0:
                        nc.sync.dma_start(dbg["d_kvp"][:], kvpack[:])
                    ag_in = pdram.tile([128, 256], BF16, tag="ag_in")
                    nc.scalar.dma_start(ag_in[:], kvpack[:])
                    ag_out = pdram.tile([NCORES * 128, 256], BF16,
                                        tag="ag_out", addr_space="Shared")
                    nc.gpsimd.collective_compute(
                        "AllGather", mybir.AluOpType.bypass,
                        replica_groups=RG,
                        ins=[ag_in[:].opt()], outs=[ag_out[:].opt()],
                    )

                    # q projection while the AG flies: head h at cols
                    # h*TOK, duplicated onto both partition halves so the
                    # even/odd-kt scores matmuls can use both PE row-halves
                    q2 = pbig.tile([128, HQ * TOK], BF16, tag="q_hb")
                    q_hb = q2[0:64, :]
                    for p in range(HT):
                        pq = pps.tile([128, 512], F32, tag="pq", bufs=4)
                        for kf in range(HT):
                            nc.tensor.matmul(
                                pq[:, 0:TOK], wq_sb[:, kf * H + p * 128:
                                                    kf * H + (p + 1) * 128],
                                h1[kf][:],
                                start=(kf == 0), stop=(kf == HT - 1),
                            )
                        nc.scalar.copy(
                            q_hb[:, (2 * p) * TOK:(2 * p + 1) * TOK],
                            pq[0:64, 0:TOK])
                        nc.vector.tensor_copy(
                            q_hb[:, (2 * p + 1) * TOK:(2 * p + 2) * TOK],
                            pq[64:128, 0:TOK])
                    rope(q_hb, HQ * TOK, 0)
                    nc.scalar.dma_start(q2[64:128, :], q2[0:64, :])
                    if DBG and l == 0:
                        nc.sync.dma_start(dbg["d_q2"][:], q2[:])

                    # unpack the gathered K/V; kt-pair c: even kt on
                    # partitions 0:64, odd kt on 64:128
                    kTt2 = pbig.tile([128, (KT // 2) * 128], BF16, tag="kT")
                    v_tok = pbig.tile([128, KT * 65], BF16, tag="v_tok")
                    if l == 0:
                        # ones columns survive across layers: the unpack DMAs
                        # only touch the V columns
                        for kt in range(KT):
                            nc.vector.tensor_copy(
                                v_tok[:, kt * 65 + 64:kt * 65 + 65],
                                ones[:, 0:1])
                    # consolidated unpack: 4 strided DMAs instead of 32
                    ag_k = ag_out[:, 0:128].rearrange("(c p) n -> p c n", c=8)
                    ag_v = ag_out[:].rearrange("(c p) n -> p c n", c=8)
                    kT3l = kTt2[0:64, :].rearrange("p (c n) -> p c n", c=8)
                    kT3h = kTt2[64:128, :].rearrange("p (c n) -> p c n", c=8)
                    vt3 = v_tok[:].rearrange("p (c n) -> p c n", c=8)
                    # K on the gpsimd queue, V on the scalar queue: the
                    # descriptor writes (~0.6us each) overlap instead of
                    # serializing on one sequencer right after the AG
                    nc.gpsimd.dma_start(kT3l, ag_k[0:64])
                    nc.gpsimd.dma_start(kT3h, ag_k[64:128])
                    nc.scalar.dma_start(vt3[:, :, 0:64], ag_v[:, :, 128:192])
                    nc.scalar.dma_start(vt3[:, :, 65:129],
                                        ag_v[:, :, 192:256])

                # attention core, software-pipelined:
                #   scores(i) -> exp(i) on ACT; attn@v(i-1) emitted after
                #   scores(i) so the PE stays back-to-back.  oT is one tile
                #   per head-pair so the o-projection starts without waiting
                #   for the last head-pair's normalization chain.
                oTs = [pacts.tile([128, TOK], BF16, tag=f"oT{k}",
                                  name=f"oT{l}_{k}")
                       for k in range(HQ // 2)]
                with tc.tile_pool(name=f"ps_att{l}", bufs=1,
                                  space="PSUM") as ppa:
                    NK = KT // 2
                    for hp in range(HQ // 2):
                        qs0 = q2[0:64, (2 * hp) * TOK:(2 * hp + 2) * TOK]
                        qs1 = q2[64:128, (2 * hp) * TOK:(2 * hp + 2) * TOK]
                        pav = ppa.tile([65, 512], F32, tag="pav", bufs=2)
                        pend = []

                        def emit_attnv(i, et):
                            nc.tensor.matmul(
                                pav[:, :],
                                v_tok[:, (2 * i) * 65:(2 * i) * 65 + 65],
                                et[:, 0:512],
                                start=(i == 0), stop=False,
                                skip_group_check=True,
                            )
                            nc.tensor.matmul(
                                pav[:, :],
                                v_tok[:, (2 * i + 1) * 65:
                                      (2 * i + 1) * 65 + 65],
                                et[:, 512:1024],
                                start=False, stop=(i == NK - 1),
                                skip_group_check=True,
                            )

                        for i in range(NK):
                            psc = ppa.tile([128, 1024], F32, tag="psc",
                                           bufs=2)
                            nc.tensor.matmul(
                                psc[:, 0:512],
                                kTt2[0:64, i * 128:(i + 1) * 128],
                                qs0, start=True, stop=True,
                            )
                            nc.tensor.matmul(
                                psc[:, 512:1024],
                                kTt2[64:128, i * 128:(i + 1) * 128],
                                qs1, start=True, stop=True,
                            )
                            et = pexp.tile([128, 1024], BF16, tag="et")
                            nc.scalar.activation(et[:], psc[:], AF.Exp)
                            pend.append((i, et))
                            if i >= 2:
                                emit_attnv(*pend.pop(0))
                        while pend:
                            emit_attnv(*pend.pop(0))

                        # normalization: fast reciprocal of the ones-row,
                        # PE broadcast, scale into oT.  (the custom-DVE op
                        # does not honor a non-zero base partition, so run
                        # it over all 65 rows and use row 64)
                        rec = psmall.tile([65, 512], F32, tag="rec", bufs=2)
                        nc.vector.reciprocal_approx_fast(rec[:], pav[:, :])
                        recr = psmall.tile([1, 512], F32R, tag="recr",
                                           bufs=2)
                        with nc.allow_low_precision(reason="f32r=fp32 bits"):
                            nc.vector.tensor_copy(recr[:], rec[64:65, :])
                        rbc = ppa.tile([64, 512], F32, tag="rbc", bufs=1)
                        nc.tensor.matmul(rbc[:], onesr[0:1, 0:64],
                                         recr[:], start=True, stop=True)
                        rbs = psmall.tile([64, 512], F32, tag="rbs", bufs=2)
                        nc.vector.tensor_copy(rbs[:], rbc[:])
                        for j in range(2):
                            nc.vector.tensor_tensor(
                                oTs[hp][j * 64:(j + 1) * 64, :],
                                pav[0:64, j * TOK:(j + 1) * TOK],
                                rbs[:, j * TOK:(j + 1) * TOK], op=MUL,
                            )

                if DBG and l == 0:
                    for k in range(HQ // 2):
                        nc.sync.dma_start(
                            dbg["d_oT"][:, k * TOK:(k + 1) * TOK], oTs[k][:])
                    nc.sync.dma_start(dbg["d_vtok"][:], v_tok[:])
                    nc.sync.dma_start(dbg["d_kT"][:], kTt2[:])
                # o projection + residual add + inline ssq for mlp rmsnorm
                with tc.tile_pool(name=f"ps_o{l}", bufs=4,
                                  space="PSUM") as ppo:
                    ssqm = ppo.tile([1, 512], F32, tag="ssqm", bufs=1)
                    for jf in range(HT):
                        po = ppo.tile([128, 512], F32, tag="po")
                        for ki in range(HT):
                            nc.tensor.matmul(
                                po[:, 0:TOK], wo_sb[:, ki * H + jf * 128:
                                                    ki * H + (jf + 1) * 128],
                                oTs[ki][:],
                                start=(ki == 0), stop=(ki == HT - 1),
                            )
                        nc.vector.tensor_tensor(
                            xs[jf][:], xs[jf][:], po[:, 0:TOK], op=ADD,
                        )
                        ssq_square(ppo, ssqm, jf, f"m{l}")
                    if DBG and l == 0:
                        for jf in range(HT):
                            nc.sync.dma_start(
                                dbg["d_x1"][:, jf * TOK:(jf + 1) * TOK],
                                xs[jf][:])
                    h2, _ = rms_tail(ppo, ssqm, f"m{l}")

                # ---------------- mlp (two-phase) ----------------
                acts = [pacts.tile([128, FPC * TOK], BF16, tag=f"act{c_}",
                                   name=f"act{l}_{c_}")
                        for c_ in range(MCH)]
                with tc.tile_pool(name=f"ps_gu{l}", bufs=2,
                                  space="PSUM") as ppg:
                    for ch in range(MCH):
                        wg_sb = pmlpw.tile([128, HT * CHF], BF16, tag="wg")
                        wu_sb = pmlpw.tile([128, HT * CHF], BF16, tag="wu")
                        nc.sync.dma_start(
                            wg_sb[:],
                            w["wg"][:, ch * HT * CHF:(ch + 1) * HT * CHF])
                        nc.sync.dma_start(
                            wu_sb[:],
                            w["wu"][:, ch * HT * CHF:(ch + 1) * HT * CHF])
                        for f in range(FPC):
                            pg = ppg.tile([128, 512], F32, tag="pg")
                            pu = ppg.tile([128, 512], F32, tag="pu")
                            for kf in range(HT):
                                rhs = h2[kf][:]
                                nc.tensor.matmul(
                                    pg[:, 0:TOK],
                                    wg_sb[:, kf * CHF + f * 128:
                                          kf * CHF + (f + 1) * 128],
                                    rhs,
                                    start=(kf == 0), stop=(kf == HT - 1),
                                )
                                nc.tensor.matmul(
                                    pu[:, 0:TOK],
                                    wu_sb[:, kf * CHF + f * 128:
                                          kf * CHF + (f + 1) * 128],
                                    rhs,
                                    start=(kf == 0), stop=(kf == HT - 1),
                                )
                            fs = slice(f * TOK, (f + 1) * TOK)
                            sil = psmall.tile([128, TOK], BF16, tag="sil",
                                              bufs=3)
                            nc.scalar.activation(sil[:], pg[:, 0:TOK],
                                                 AF.Silu)
                            nc.vector.tensor_tensor(
                                acts[ch][:, fs], sil[:], pu[:, 0:TOK],
                                op=MUL,
                            )

                if DBG and l == 0:
                    for c_ in range(MCH):
                        nc.sync.dma_start(
                            dbg["d_act"][:, c_ * FPC * TOK:
                                         (c_ + 1) * FPC * TOK], acts[c_][:])

                # phase B: down-proj, f-outer so all 8 output tiles
                # accumulate in PSUM across the full FF dimension
                with tc.tile_pool(name=f"ps_d{l}", bufs=1,
                                  space="PSUM") as ppd:
                    pds = [ppd.tile([128, 512], F32, tag=f"pd{jf}", bufs=1,
                                    name=f"pd{l}_{jf}")
                           for jf in range(HT)]
                    FPW = 4   # f-tiles per wd DMA
                    for f in range(FT):
                        if f % FPW == 0:
                            wd_sb = pdw.tile([128, FPW * H], BF16, tag="wd")
                            nc.sync.dma_start(
                                wd_sb[:],
                                w["wd"][:, f * H:(f + FPW) * H])
                            if f == FPW and l + 1 < L:
                                # prefetch next layer's attention weights
                                # (after the first wd tiles so phase B is
                                # not starved behind them on the queue)
                                attw_cur = load_attw(l + 1)
                        for jf in range(HT):
                            nc.tensor.matmul(
                                pds[jf][:, 0:TOK],
                                wd_sb[:, (f % FPW) * H + jf * 128:
                                      (f % FPW) * H + (jf + 1) * 128],
                                acts[f // FPC][:, (f % FPC) * TOK:
                                               (f % FPC + 1) * TOK],
                                start=(f == 0), stop=(f == FT - 1),
                                skip_group_check=True,
                            )
                    for jf in range(HT):
                        nc.vector.tensor_tensor(
                            xs[jf][:], xs[jf][:], pds[jf][:, 0:TOK], op=ADD,
                        )
                        if l + 1 < L:
                            pass  # ssq emitted below (needs a free bank)
                        else:
                            nc.sync.dma_start(
                                xout[:, jf * TOK:(jf + 1) * TOK], xs[jf][:],
                            )

                if l + 1 < L:
                    with tc.tile_pool(name=f"ps_n{l + 1}", bufs=1,
                                      space="PSUM") as ppn:
                        ssqn = ppn.tile([1, 512], F32, tag="ssqn")
                        for kf in range(HT):
                            ssq_square(ppn, ssqn, kf, f"a{l + 1}")
                        h1, rb1 = rms_tail(ppn, ssqn, f"a{l + 1}")
    # populate .instr bytes for extended-inst InstISA subclasses (the custom
    # DVE reciprocal); raw Bass does not run this pass automatically
    mybir.codegen_inst_isa_subclasses(nc)
    return nc


def _get_nc():
    global _CACHED_NC
    if _CACHED_NC is None:
        _CACHED_NC = _build_nc()
    return _CACHED_NC


def _pack_lhsT(wT, kt, m):
    """[K, M] lhsT -> SBUF image [128, kt*m] (block kf at cols kf*m)."""
    return np.ascontiguousarray(
        wT.reshape(kt, 128, m).transpose(1, 0, 2).reshape(128, kt * m))


def _host_prep(inputs):
    """Fold ln/scale into weights, build SBUF-image layouts, embed gather,
    rope tables.  Returns in_maps (list of dicts, one per core)."""
    ids = np.asarray(inputs["input_ids"])[0]          # [S] int32
    embed = np.asarray(inputs["embed"], np.float32)   # [V, H]
    x = embed[ids]                                    # [S, H]

    inv = 1.0 / (10000.0 ** (np.arange(0, D, 2, dtype=np.float32) / D))
    freqs = np.arange(S, dtype=np.float32)[:, None] * inv[None, :]   # [S,32]
    cosT = np.cos(freqs).T.astype(np.float32)   # [32, S]
    sinT = np.sin(freqs).T.astype(np.float32)
    cosF = np.tile(cosT, (2, 1))                             # [64, S]
    sinF = np.concatenate([-sinT, sinT], 0)                  # [64, S]

    def bf(a):
        return np.ascontiguousarray(a).astype(BF)

    scale = np.float32(1.0 / np.sqrt(D))
    shared = {}
    for l in range(L):
        ln1 = np.asarray(inputs["ln1"], np.float32)[l]
        ln2 = np.asarray(inputs["ln2"], np.float32)[l]
        wq = np.asarray(inputs["Wq"], np.float32)[l] * ln1[None, :] * scale
        wk = np.asarray(inputs["Wk"], np.float32)[l] * ln1[None, :]
        wv = np.asarray(inputs["Wv"], np.float32)[l] * ln1[None, :]
        wo = np.asarray(inputs["Wo"], np.float32)[l]
        wg = np.asarray(inputs["Wg"], np.float32)[l] * ln2[None, :]
        wu = np.asarray(inputs["Wu"], np.float32)[l] * ln2[None, :]
        wd = np.asarray(inputs["Wd"], np.float32)[l]
        shared[f"wq{l}"] = bf(_pack_lhsT(wq.T, HT, H))
        shared[f"wkv{l}"] = np.ascontiguousarray(
            _pack_lhsT(np.concatenate([wk, wv], 0).T, HT, 128)
        ).astype(np.float32)
        shared[f"wo{l}"] = bf(_pack_lhsT(wo.T, HT, H))
        # wg/wu: chunk-major: [128, ch(8) x kf(8) x 512]
        wgT = wg.T.reshape(HT, 128, MCH, CHF)     # [kf, part, ch, chf]
        shared[f"wg{l}"] = bf(
            wgT.transpose(1, 2, 0, 3).reshape(128, MCH * HT * CHF))
        wuT = wu.T.reshape(HT, 128, MCH, CHF)
        shared[f"wu{l}"] = bf(
            wuT.transpose(1, 2, 0, 3).reshape(128, MCH * HT * CHF))
        shared[f"wd{l}"] = bf(_pack_lhsT(wd.T, FT, H))

    in_maps = []
    for c in range(NCORES):
        cs = slice(c * TOK, (c + 1) * TOK)
        xc = x[cs, :].T                                    # [H, TOK]
        m = {
            "x0": np.ascontiguousarray(
                xc.reshape(HT, 128, TOK).transpose(1, 0, 2).reshape(
                    128, HT * TOK)),
            "cosq": bf(np.tile(cosF[:, cs], (1, HPG))),
            "sinq": bf(np.tile(sinF[:, cs], (1, HPG))),
        }
        m.update(shared)
        in_maps.append(m)
    return in_maps


def kernel(**inputs) -> np.ndarray:
    nc = _get_nc()
    in_maps = _host_prep(inputs)
    res = bass_utils.run_bass_kernel_spmd(
        nc, in_maps, core_ids=list(range(NCORES))
    )
    out = np.empty((1, S, H), np.float32)
    for c in range(NCORES):
        xo = res.results[c]["xout"]                 # [128, HT*TOK]
        xc = xo.reshape(128, HT, TOK).transpose(1, 0, 2).reshape(H, TOK)
        out[0, c * TOK:(c + 1) * TOK, :] = xc.T
    return out
